# revision 27
# baseline (speedup 1.0000x reference)
"""Trainium2 Bass kernel for nn_LiveRiskModel (hierarchical transformer).

Sharding: pure data-parallel over B=8 (one batch element per NeuronCore).

Patch-encoder attention exploits structure:
- The patch-encoder query is the CLS token, whose input embedding is the
  constant patch_cls vector -> q is identical for every patch and is
  precomputed on host.
- Scores for all 2112 tokens of a chunk are one row-matmul (q stationary),
  scattered token-major via an SBUF->SBUF DMA, exp'd, then expanded to a
  block-diagonal [99,64] weight matrix per 3-patch group with a single
  tensor_scalar multiply against a host-built 0/1 mask.
- V is produced directly token-major (X0 token-block as the stationary
  operand of the QKV matmul), so A@V needs no transposes; softmax
  denominators accumulate via a free-dim-1 matmul against ones, and the
  normalization folds into the av^T eviction scale. V's bias is added
  after the transpose back to d-major (softmax weights sum to 1).
- CLS residual of the patch encoder is the constant patch_cls -> folded
  into the out-proj bias on host.
"""
import sys

sys.path.insert(0, "/opt/trn_rl_repo")

import numpy as np
import ml_dtypes

import concourse.bass as bass
import concourse.mybir as mybir
import concourse.tile as tile
from concourse import bacc
from concourse.bass_utils import run_bass_kernel_spmd
from concourse.masks import make_identity

F32 = mybir.dt.float32
F32R = mybir.dt.float32r
F16 = mybir.dt.float16
BF16 = mybir.dt.bfloat16
FP8 = mybir.dt.float8e4
AF = mybir.ActivationFunctionType
ALU = mybir.AluOpType
AX = mybir.AxisListType

B, U, T, L = 8, 16, 16, 32
E, FF, DESC, AE, DE = 256, 2048, 768, 64, 128
NPATCH = 256            # per core
SP = 33                 # patch seq len (CLS + 32)
NCHUNK, PCH = 4, 64     # patch chunks per core, patches per chunk
TOKC = PCH * SP         # 2112 tokens per chunk
NG = 16                 # 4-patch (128-token) groups per chunk; CLS out-of-band
SM = 257                # main seq len
KCH = [(0, 128), (128, 128), (256, 1)]   # main seq chunking

_CACHE = {}


def restride(ap, pairs):
    return bass.AP(ap.tensor, ap.offset, list(ap.ap[: len(ap.ap) - len(pairs)]) + pairs)


def _emit(nc):
    # ---------------- DRAM I/O ----------------
    descT = nc.dram_tensor("descT", [128, 6, 8192], FP8, kind="ExternalInput")
    extras = nc.dram_tensor("extras", [3, 8192], BF16, kind="ExternalInput")
    pcls = nc.dram_tensor("pcls", [128, 2], F32, kind="ExternalInput")
    mcls = nc.dram_tensor("mcls", [128, 2], F32, kind="ExternalInput")
    posb = nc.dram_tensor("posb", [128, 2 * 256], F32, kind="ExternalInput")
    w_emb = nc.dram_tensor("w_emb", [128, 6 * E], FP8, kind="ExternalInput")
    w_ex = nc.dram_tensor("w_ex", [3, E], BF16, kind="ExternalInput")
    ucls = nc.dram_tensor("ucls", [128, 2 * 2], F16, kind="ExternalInput")
    amask = nc.dram_tensor("amask", [128, 124], F16, kind="ExternalInput")
    ecv = nc.dram_tensor("ecv", [1, 2 * 129], F16, kind="ExternalInput")
    pe_wqkv = nc.dram_tensor("pe_wqkv", [128, 2 * 3 * E], F16, kind="ExternalInput")
    pe_bqkv = nc.dram_tensor("pe_bqkv", [128, 6], F32, kind="ExternalInput")
    pe_wo = nc.dram_tensor("pe_wo", [128, 2 * E], F16, kind="ExternalInput")
    pe_bo = nc.dram_tensor("pe_bo", [128, 2], F32, kind="ExternalInput")
    pe_g1 = nc.dram_tensor("pe_g1", [128, 2], F32, kind="ExternalInput")
    pe_b1v = nc.dram_tensor("pe_b1v", [128, 2], F32, kind="ExternalInput")
    pe_w1 = nc.dram_tensor("pe_w1", [128, 2 * FF], F16, kind="ExternalInput")
    pe_b1 = nc.dram_tensor("pe_b1", [128, 16], F32, kind="ExternalInput")
    pe_w2 = nc.dram_tensor("pe_w2", [128, 16 * E], F16, kind="ExternalInput")
    pe_b2 = nc.dram_tensor("pe_b2", [128, 2], F32, kind="ExternalInput")
    pe_g2 = nc.dram_tensor("pe_g2", [128, 2], F32, kind="ExternalInput")
    pe_b2v = nc.dram_tensor("pe_b2v", [128, 2], F32, kind="ExternalInput")
    mt_wqkv = nc.dram_tensor("mt_wqkv", [128, 2 * 2 * 3 * E], F16, kind="ExternalInput")
    mt_bqkv = nc.dram_tensor("mt_bqkv", [128, 6 * 2], F32, kind="ExternalInput")
    mt_wo = nc.dram_tensor("mt_wo", [128, 2 * 2 * E], F16, kind="ExternalInput")
    mt_bo = nc.dram_tensor("mt_bo", [128, 2 * 2], F32, kind="ExternalInput")
    mt_g1 = nc.dram_tensor("mt_g1", [128, 2 * 2], F32, kind="ExternalInput")
    mt_b1v = nc.dram_tensor("mt_b1v", [128, 2 * 2], F32, kind="ExternalInput")
    mt_w1 = nc.dram_tensor("mt_w1", [128, 2 * 2 * FF], F16, kind="ExternalInput")
    mt_b1 = nc.dram_tensor("mt_b1", [128, 16 * 2], F32, kind="ExternalInput")
    mt_w2 = nc.dram_tensor("mt_w2", [128, 16 * 2 * E], F16, kind="ExternalInput")
    mt_b2 = nc.dram_tensor("mt_b2", [128, 2 * 2], F32, kind="ExternalInput")
    mt_g2 = nc.dram_tensor("mt_g2", [128, 2 * 2], F32, kind="ExternalInput")
    mt_b2v = nc.dram_tensor("mt_b2v", [128, 2 * 2], F32, kind="ExternalInput")
    w_head = nc.dram_tensor("w_head", [128, 2 * 2], F32, kind="ExternalInput")
    b_head = nc.dram_tensor("b_head", [1, 2], F32, kind="ExternalInput")
    out = nc.dram_tensor("out", [1, 2], F32, kind="ExternalOutput")
    s_dram = nc.dram_tensor("s_scratch", [2, 2, 2048], F16, kind="Internal")

    import contextlib
    with tile.TileContext(nc) as tc, contextlib.ExitStack() as es:
        wp = es.enter_context(tc.tile_pool(name="wp", bufs=1))
        cp = es.enter_context(tc.tile_pool(name="cp", bufs=2))
        sp_ = es.enter_context(tc.tile_pool(name="sp", bufs=3))
        ap_ = es.enter_context(tc.tile_pool(name="ap", bufs=4))
        ep = es.enter_context(tc.tile_pool(name="ep", bufs=2))
        vp1 = es.enter_context(tc.tile_pool(name="vp1", bufs=1))
        fp = es.enter_context(tc.tile_pool(name="fp", bufs=1))
        srp = es.enter_context(tc.tile_pool(name="srp", bufs=1))
        lp = es.enter_context(tc.tile_pool(name="lp", bufs=2))
        mp = es.enter_context(tc.tile_pool(name="mp", bufs=1))
        ppA = es.enter_context(tc.tile_pool(name="ppA", bufs=3, space="PSUM"))
        ppS = es.enter_context(tc.tile_pool(name="ppS", bufs=2, space="PSUM"))
        ppC = es.enter_context(tc.tile_pool(name="ppC", bufs=2, space="PSUM"))

        # ---------------- weights -> SBUF ----------------
        idf32 = wp.tile([128, 128], F32)
        make_identity(nc, idf32)
        idf16 = wp.tile([128, 128], F16)
        make_identity(nc, idf16)
        eps_sb = wp.tile([128, 1], F32)
        nc.vector.memset(eps_sb[:], 1e-5)
        ones64 = wp.tile([1, 64], F16)
        nc.vector.memset(ones64[:], 1.0)
        onescol = wp.tile([128, 1], F32R, name="onescol")
        nc.vector.memset(onescol[:].bitcast(F32), 1.0)
        onesrow = wp.tile([1, 128], F32R, name="onesrow")
        nc.vector.memset(onesrow[:].bitcast(F32), 1.0)

        def load(name, dram, shape, dt):
            t = wp.tile(shape, dt, name=name)
            flat = t[:] if len(shape) <= 2 else t[:].rearrange(
                {3: "p a b -> p (a b)", 4: "p a b c -> p (a b c)"}[len(shape)])
            src = dram[:] if dt != F32R else dram[:].bitcast(F32R)
            nc.gpsimd.dma_start(flat, src)
            return t

        w_emb_sb = load("w_emb_sb", w_emb, [128, 6, E], FP8)
        w_ex_sb = load("w_ex_sb", w_ex, [3, E], BF16)
        ucls_sb = load("ucls_sb", ucls, [128, 2, 2], F16)
        amask_sb = load("amask_sb", amask, [128, 124], F16)
        ecv_sb = load("ecv_sb", ecv, [1, 2, 129], F16)
        pe_wqkv_sb = load("pe_wqkv_sb", pe_wqkv, [128, 2, 3 * E], F16)
        pe_bqkv_sb = load("pe_bqkv_sb", pe_bqkv, [128, 6], F32)
        pe_wo_sb = load("pe_wo_sb", pe_wo, [128, 2, E], F16)
        pe_bo_sb = load("pe_bo_sb", pe_bo, [128, 2], F32)
        pe_g1_sb = load("pe_g1_sb", pe_g1, [128, 2], F32)
        pe_b1v_sb = load("pe_b1v_sb", pe_b1v, [128, 2], F32)
        pe_w1_sb = load("pe_w1_sb", pe_w1, [128, 2, FF], F16)
        pe_b1_sb = load("pe_b1_sb", pe_b1, [128, 16], F32)
        pe_w2_sb = load("pe_w2_sb", pe_w2, [128, 16, E], F16)
        pe_b2_sb = load("pe_b2_sb", pe_b2, [128, 2], F32)
        pe_g2_sb = load("pe_g2_sb", pe_g2, [128, 2], F32)
        pe_b2v_sb = load("pe_b2v_sb", pe_b2v, [128, 2], F32)
        pcls_sb = load("pcls_sb", pcls, [128, 2, 1], F32)

        x2clsT = wp.tile([128, 2, 256], F16, name="x2clsT")  # patch-enc CLS outputs
        av_all = wp.tile([128, 2, 256], F16, name="av_all")  # patch attn out, d-major

        # ---------------- LN helper ----------------
        def ln_std(tag, z, col0, qcs, out_dt, out_m_aps, gam, bet, gi=None):
            """standardize z[:, :, col0:col0+qcs] (f32, [128,2,*]) over E=256,
            apply per-channel affine on the transposed-back evict into
            out_m_aps[m] ([128, qcs] APs, dtype out_dt)."""
            ztm = lp.tile([128, 256], F32, name="ln_z")
            for m in range(2):
                tp = ppA.tile([128, 352], F32, name="big")
                zs = z[:, m, col0:col0 + qcs]
                zs = zs.bitcast(F32) if zs.dtype == F32R else zs
                nc.tensor.transpose(tp[:qcs, :128], zs, idf32[:])
                nc.vector.tensor_copy(ztm[:qcs, m * 128:(m + 1) * 128], tp[:qcs, :128])
            mu = lp.tile([128, 1], F32, name="ln_mu")
            nc.vector.reduce_sum(mu[:qcs], ztm[:qcs], axis=AX.X)
            nc.scalar.mul(mu[:qcs], mu[:qcs], 1.0 / 256.0)
            sq = lp.tile([128, 256], F32, name="ln_sq")
            ssq = lp.tile([128, 1], F32, name="ln_ssq")
            nc.scalar.activation(sq[:qcs], ztm[:qcs], AF.Square, accum_out=ssq[:qcs])
            musq = lp.tile([128, 1], F32, name="ln_musq")
            nc.scalar.square(musq[:qcs], mu[:qcs])
            var = lp.tile([128, 1], F32, name="ln_var")
            nc.vector.scalar_tensor_tensor(var[:qcs], ssq[:qcs], 1.0 / 256.0, musq[:qcs], ALU.mult, ALU.subtract)
            std = lp.tile([128, 1], F32, name="ln_std")
            nc.scalar.activation(std[:qcs], var[:qcs], AF.Sqrt, bias=eps_sb[:qcs])
            rstd = lp.tile([128, 1], F32, name="ln_rstd")
            nc.vector.reciprocal(rstd[:qcs], std[:qcs])
            xh = lp.tile([128, 256], F32, name="ln_xh")
            nc.vector.tensor_scalar(xh[:qcs], ztm[:qcs], mu[:qcs], rstd[:qcs], ALU.subtract, ALU.mult)
            for m in range(2):
                tp = ppA.tile([128, 352], F32, name="big")
                nc.tensor.transpose(tp[:128, :qcs], xh[:qcs, m * 128:(m + 1) * 128], idf32[:qcs, :qcs])
                g = gam[:, m:m + 1] if gi is None else gam[:, m:m + 1, gi]
                b = bet[:, m:m + 1] if gi is None else bet[:, m:m + 1, gi]
                nc.scalar.activation(out_m_aps[m], tp[:, :qcs], AF.Identity, bias=b, scale=g)

        def ln_fast(z, ntok, out_m_aps, gam, bet, gi=None):
            """LN over E=256 without transposes: partition sums via ones-
            stationary matmuls, mean/rstd broadcast back via replicate matmul.
            z: [128, 2, ntok] f32r sbuf tile."""
            nte = ntok + (ntok % 2)   # fp32r MMs need even free dim
            sq = lp.tile([128, 2, 264], F32R, name="lf_sq")
            for m in range(2):
                with nc.allow_low_precision(reason="f32r LN"):
                    nc.scalar.square(sq[:, m, :nte], z[:, m, :nte].bitcast(F32))
            sums = ppA.tile([128, 352], F32, name="big")
            sums2 = ppS.tile([128, 352], F32, name="srow")
            for m in range(2):
                nc.tensor.matmul(sums[:1, :nte], onescol[:],
                                 z[:, m, :nte], start=(m == 0), stop=(m == 1))
                nc.tensor.matmul(sums2[:1, :nte], onescol[:],
                                 sq[:, m, :nte], start=(m == 0), stop=(m == 1))
            mu = lp.tile([1, 264], F32R, name="lf_mu")
            with nc.allow_low_precision(reason="f32r LN"):
                nc.scalar.mul(mu[:, :nte], sums[:1, :nte], 1.0 / 256.0)
            musq = lp.tile([1, 264], F32, name="lf_musq")
            nc.scalar.square(musq[:, :ntok], mu[:, :ntok].bitcast(F32))
            var = lp.tile([1, 264], F32, name="lf_var")
            nc.vector.scalar_tensor_tensor(var[:, :ntok], sums2[:1, :ntok], 1.0 / 256.0,
                                           musq[:, :ntok], ALU.mult, ALU.subtract)
            sd = lp.tile([1, 264], F32, name="lf_sd")
            nc.scalar.activation(sd[:, :ntok], var[:, :ntok], AF.Sqrt, bias=eps_sb[:1])
            rs = lp.tile([1, 264], F32R, name="lf_rs")
            with nc.allow_low_precision(reason="f32r LN"):
                nc.vector.reciprocal(rs[:, :nte].bitcast(F32).bitcast(F32R), sd[:, :nte])
            MU = ppA.tile([128, 352], F32, name="big")
            nc.tensor.matmul(MU[:, :nte], onesrow[:],
                             mu[:, :nte], start=True, stop=True)
            RS = ppS.tile([128, 352], F32, name="srow")
            nc.tensor.matmul(RS[:, :nte], onesrow[:],
                             rs[:, :nte], start=True, stop=True)
            for m in range(2):
                t1 = lp.tile([128, 264], F32, name="lf_t1")
                nc.vector.tensor_tensor(t1[:, :ntok], z[:, m, :ntok].bitcast(F32), MU[:, :ntok], ALU.subtract)
                nc.vector.tensor_tensor(t1[:, :ntok], t1[:, :ntok], RS[:, :ntok], ALU.mult)
                g = gam[:, m:m + 1] if gi is None else gam[:, m:m + 1, gi]
                b = bet[:, m:m + 1] if gi is None else bet[:, m:m + 1, gi]
                nc.scalar.activation(out_m_aps[m], t1[:, :ntok], AF.Identity, bias=b, scale=g)

        # ================= PATCH PHASE =================
        # Software pipeline: stage A(c) = embed + K + V_tm + scores + scatter
        # + exp; stage B(c) = mask-expand + AV/den + avT->av. Emission order
        # A(0), A(1), B(0), A(2), B(1), A(3), B(2), B(3) keeps TensorE fed
        # while the score->exp chain of the previous chunk completes.
        def grp(g):
            return 128 * g, 128

        def stage_a(c):
            X0T = cp.tile([128, 2, 2048], F16, name="X0T")
            e_exp = ep.tile([128, 2, NG], F32, name="e_exp")

            # ---- embed: X0T = desc @ Wcomb.T + extras (ids/anchor/bias) ----
            groups = [(16 * t, 16) for t in range(4)]
            dr = er = None
            dro = 0
            for gi_, (p0, npat) in enumerate(groups):
                W = 512
                if gi_ % 2 == 0:
                    dsl2 = slice(c * 2048 + 32 * p0, c * 2048 + 32 * p0 + 1024)
                    dr = sp_.tile([128, 6, 1024], FP8, name="dr")
                    nc.sync.dma_start(dr[:], descT[:, :, dsl2])
                    er = sp_.tile([3, 1024], BF16, name="er")
                    nc.sync.dma_start(er[:], extras[:, dsl2])
                    dro = 0
                for m in range(2):
                    ps = ppA.tile([128, 512], F32, name="big")
                    for kc in range(3):
                        nc.tensor.matmul(ps[:, :W], w_emb_sb[:, 2 * kc:2 * kc + 2, m * 128:(m + 1) * 128],
                                         dr[:, 2 * kc:2 * kc + 2, dro:dro + W],
                                         start=(kc == 0), stop=False,
                                         perf_mode=mybir.MatmulPerfMode.DoubleRow)
                    nc.tensor.matmul(ps[:, :W], w_ex_sb[:, m * 128:(m + 1) * 128],
                                     er[:, dro:dro + W], start=False, stop=True)
                    oap = X0T[:, m, 32 * p0:32 * p0 + W]
                    if (gi_ + m) % 2 == 0:
                        nc.scalar.mul(oap, ps[:, :W], 1.0 / 64.0)
                    else:
                        nc.vector.tensor_scalar_mul(oap, ps[:, :W], 1.0 / 64.0)
                dro += W

            # ---- scores: s = u.x0 (u = Wk^T q host-folded; bias const cancels) ----
            s_row = srp.tile([1, 2, 2048], F16, name="s_row")
            stiles = [(0, 512), (512, 512), (1024, 512), (1536, 512)]
            for h in range(2):
                for t, (c0, w) in enumerate(stiles):
                    csl = slice(c0, c0 + w)
                    ps = ppS.tile([128, 512], F32, name="srow")
                    for kc in range(2):
                        nc.tensor.matmul(ps[:1, :w], ucls_sb[:, kc, h:h + 1],
                                         X0T[:, kc, csl], start=(kc == 0), stop=(kc == 1))
                    if t % 2 == 0:
                        nc.scalar.copy(s_row[:, h, csl], ps[:1, :w])
                    else:
                        nc.vector.tensor_copy(s_row[:, h, csl], ps[:1, :w])
            e_sc = ep.tile([128, 2, NG], F16, name="e_sc")
            for h in range(2):
                # token-major scatter via DRAM bounce (1-part row -> [128, NG])
                nc.sync.dma_start(s_dram[c % 2, h, :], s_row[:, h, :])
                d0 = s_dram[c % 2, h, :]
                nc.sync.dma_start(
                    e_sc[:, h, :],
                    bass.AP(d0.tensor, d0.offset, [[1, 128], [128, NG]]))
                nc.scalar.activation(e_exp[:, h, :], e_sc[:, h, :], AF.Exp)
            return X0T, e_exp

        def stage_b(c, tiles):
            X0T, e_exp = tiles
            # ---- V token-major (no bias; bias added post-attention) ----
            Vtm = vp1.tile([128, NG, 2, 129], F16, name="Vtm")
            nc.vector.memset(Vtm[:, :, :, 128:129], 1.0)
            for g in range(NG):
                t0, ntk = grp(g)
                ps = ppA.tile([128, 512], F32, name="big")
                for kc in range(2):
                    nc.tensor.matmul(ps[:ntk, :256], X0T[:, kc, t0:t0 + ntk],
                                     pe_wqkv_sb[:, kc, 4 * 128:6 * 128],
                                     start=(kc == 0), stop=(kc == 1))
                nc.scalar.copy(Vtm[:ntk, g, 0, :128], ps[:ntk, :128])
                nc.vector.tensor_copy(Vtm[:ntk, g, 1, :128], ps[:ntk, 128:256])
            for h in range(2):
                AVp = ppC.tile([128, 352], F32, name="avp")
                for g in range(NG):
                    t0, ntk = grp(g)
                    E3m = ap_.tile([128, 64], F16, name="e3m")
                    nc.vector.tensor_scalar_mul(
                        E3m[:ntk, :], amask_sb[:ntk, 60 - 4 * g:124 - 4 * g],
                        e_exp[:ntk, h, g:g + 1])
                    nc.tensor.matmul(AVp[:64, :129], E3m[:ntk, :], Vtm[:ntk, g, h, :],
                                     start=(g == 0), stop=False)
                # CLS token contribution: constant e_cls * [v_cls, 1] per head
                nc.tensor.matmul(AVp[:64, :129], ones64[:], ecv_sb[:, h, :],
                                 start=False, stop=True)
                rc = ap_.tile([64, 1], F32, name="rc")
                nc.vector.reciprocal(rc[:], AVp[:64, 128:129])
                avT = ap_.tile([64, 128], F32, name="avT")
                nc.scalar.activation(avT[:], AVp[:64, :128], AF.Copy, scale=rc[:])
                tp = ppS.tile([128, 352], F32, name="srow")
                nc.tensor.transpose(tp[:, :64], avT[:], idf32[:64, :64])
                nc.scalar.activation(av_all[:, h, c * 64:(c + 1) * 64], tp[:, :64],
                                     AF.Identity, bias=pe_bqkv_sb[:, 4 + h:5 + h])

        tiles = [None] * NCHUNK
        tiles[0] = stage_a(0)
        for c in range(NCHUNK):
            if c + 1 < NCHUNK:
                tiles[c + 1] = stage_a(c + 1)
            stage_b(c, tiles[c])
            tiles[c] = None

        # main-phase weights: loaded only now so startup DMA doesn't block embed
        posb_sb = load("posb_sb", posb, [128, 2, 256], F32)
        mcls_sb = load("mcls_sb", mcls, [128, 2, 1], F32)
        mt_wqkv_sb = load("mt_wqkv_sb", mt_wqkv, [128, 2, 2, 3 * E], F16)
        mt_bqkv_sb = load("mt_bqkv_sb", mt_bqkv, [128, 6, 2], F32)
        mt_wo_sb = load("mt_wo_sb", mt_wo, [128, 2, 2, E], F16)
        mt_bo_sb = load("mt_bo_sb", mt_bo, [128, 2, 2], F32)
        mt_g1_sb = load("mt_g1_sb", mt_g1, [128, 2, 2], F32)
        mt_b1v_sb = load("mt_b1v_sb", mt_b1v, [128, 2, 2], F32)
        mt_w1_sb = load("mt_w1_sb", mt_w1, [128, 2, 2, FF], F16)
        mt_b1_sb = load("mt_b1_sb", mt_b1, [128, 16, 2], F32)
        mt_w2_sb = load("mt_w2_sb", mt_w2, [128, 16, 2, E], F16)
        mt_b2_sb = load("mt_b2_sb", mt_b2, [128, 2, 2], F32)
        mt_g2_sb = load("mt_g2_sb", mt_g2, [128, 2, 2], F32)
        mt_b2v_sb = load("mt_b2v_sb", mt_b2v, [128, 2, 2], F32)
        w_head_sb = load("w_head_sb", w_head, [128, 2, 2], F32)
        b_head_sb = load("b_head_sb", b_head, [1, 2], F32)

        # ---- out-proj + z1 + LN1 (all 256 CLS at once) ----
        z1c = fp.tile([128, 2, 256], F32R, name="z1c")
        x1c = fp.tile([128, 2, 256], F16, name="x1c")
        for m in range(2):
            ps = ppA.tile([128, 256], F32, name="big")
            for kc in range(2):
                nc.tensor.matmul(ps[:], pe_wo_sb[:, kc, m * 128:(m + 1) * 128],
                                 av_all[:, kc, :], start=(kc == 0), stop=(kc == 1))
            # pe_bo has patch_cls folded in (CLS residual)
            nc.scalar.activation(z1c[:, m, :], ps[:], AF.Identity, bias=pe_bo_sb[:, m:m + 1])
        ln_fast(z1c, 256, [x1c[:, 0, :], x1c[:, 1, :]], pe_g1_sb, pe_b1v_sb)

        # ---- FFN (all 256 CLS) ----
        Hc = fp.tile([128, 16, 256], F16, name="Hc")
        z2c = fp.tile([128, 2, 256], F32R, name="z2c")
        for fm in range(16):
            ps = ppA.tile([128, 256], F32, name="big")
            for kc in range(2):
                nc.tensor.matmul(ps[:], pe_w1_sb[:, kc, fm * 128:(fm + 1) * 128],
                                 x1c[:, kc, :], start=(kc == 0), stop=(kc == 1))
            nc.scalar.activation(Hc[:, fm, :], ps[:], AF.Relu, bias=pe_b1_sb[:, fm:fm + 1])
        for m in range(2):
            ps = ppA.tile([128, 256], F32, name="big")
            for fk in range(16):
                nc.tensor.matmul(ps[:], pe_w2_sb[:, fk, m * 128:(m + 1) * 128],
                                 Hc[:, fk, :], start=(fk == 0), stop=(fk == 15))
            nc.vector.scalar_tensor_tensor(z2c[:, m, :], ps[:], pe_b2_sb[:, m:m + 1],
                                           x1c[:, m, :], ALU.add, ALU.add)
        ln_fast(z2c, 256, [x2clsT[:, 0, :], x2clsT[:, 1, :]], pe_g2_sb, pe_b2v_sb)

        # ================= MAIN PHASE =================
        xin = mp.tile([128, 2, SM], F16, name="xm0")
        for m in range(2):
            nc.vector.tensor_add(xin[:, m, 1:], x2clsT[:, m, :], posb_sb[:, m, :])
        nc.scalar.copy(xin[:, :, 0:1], mcls_sb[:])

        QTm = mp.tile([128, 2, SM], F16, name="QTm")
        KTm = mp.tile([128, 2, SM], F16, name="KTm")
        VTm = mp.tile([128, 2, SM], F32, name="VTm")
        Vtm_k = [mp.tile([kcs, 4, 65], F16, name=f"Vtm{kc}") for kc, (c0, kcs) in enumerate(KCH)]
        ET_k = [mp.tile([kcs, 4, SM], F16, name=f"ET{kc}") for kc, (c0, kcs) in enumerate(KCH)]
        Otm_k = [mp.tile([kcs, 4, 64], F32, name=f"Otm{kc}") for kc, (c0, kcs) in enumerate(KCH)]

        for li in range(2):
            cls_only = (li == 1)
            nq = 1 if cls_only else SM
            # ---- QKV ----
            mrange = [0, 1, 2, 3, 4, 5] if not cls_only else [2, 3, 4, 5]
            for m in mrange:
                ps = ppA.tile([128, 352], F32, name="big")
                for kc in range(2):
                    nc.tensor.matmul(ps[:, :SM], mt_wqkv_sb[:, kc, li, m * 128:(m + 1) * 128],
                                     xin[:, kc, :], start=(kc == 0), stop=(kc == 1))
                if m < 2:
                    dst = QTm[:, m, :]
                elif m < 4:
                    dst = KTm[:, m - 2, :]
                else:
                    dst = VTm[:, m - 4, :]
                nc.scalar.activation(dst, ps[:, :SM], AF.Identity, bias=mt_bqkv_sb[:, m:m + 1, li])
            if cls_only:
                for m in range(2):
                    ps = ppA.tile([128, 352], F32, name="big")
                    for kc in range(2):
                        nc.tensor.matmul(ps[:, :1], mt_wqkv_sb[:, kc, li, m * 128:(m + 1) * 128],
                                         xin[:, kc, 0:1], start=(kc == 0), stop=(kc == 1))
                    nc.scalar.activation(QTm[:, m, 0:1], ps[:, :1], AF.Identity,
                                         bias=mt_bqkv_sb[:, m:m + 1, li])
            # ---- V token-major (+ones col) ----
            for kc, (c0, kcs) in enumerate(KCH):
                for h in range(4):
                    hm, hr = divmod(h, 2)
                    tp = ppA.tile([128, 352], F32, name="big")
                    nc.tensor.transpose(tp[:kcs, :64], VTm[hr * 64:hr * 64 + 64, hm, c0:c0 + kcs],
                                        idf32[hr * 64:hr * 64 + 64, hr * 64:hr * 64 + 64])
                    nc.vector.tensor_copy(Vtm_k[kc][:kcs, h, :64], tp[:kcs, :64])
                nc.vector.memset(Vtm_k[kc][:kcs, :, 64:65], 1.0)

            if not cls_only:
                # ---- full attention ----
                for h in range(4):
                    hm, hr = divmod(h, 2)
                    KTh = KTm[hr * 64:hr * 64 + 64, hm, :]
                    QTh = QTm[hr * 64:hr * 64 + 64, hm, :]
                    for kc, (c0, kcs) in enumerate(KCH):
                        ps = ppA.tile([128, 352], F32, name="big")
                        nc.tensor.matmul(ps[:kcs, :SM], KTh[:, c0:c0 + kcs], QTh, start=True, stop=True)
                        nc.scalar.activation(ET_k[kc][:kcs, h, :], ps[:kcs, :SM], AF.Exp)
                for h in range(4):
                    for qc, (q0, qcs) in enumerate(KCH):
                        op = ppC.tile([128, 352], F32, name="avp")
                        for kc, (c0, kcs) in enumerate(KCH):
                            nc.tensor.matmul(op[:qcs, :65], ET_k[kc][:kcs, h, q0:q0 + qcs],
                                             Vtm_k[kc][:kcs, h, :], start=(kc == 0), stop=(kc == 2))
                        rc = ap_.tile([128, 1], F32, name="rcm")
                        nc.vector.reciprocal(rc[:qcs], op[:qcs, 64:65])
                        nc.scalar.activation(Otm_k[qc][:qcs, h, :], op[:qcs, :64], AF.Copy, scale=rc[:qcs])
                aOTm = mp.tile([128, 2, SM], F16, name="aOTm")
                for qc, (q0, qcs) in enumerate(KCH):
                    for m in range(2):
                        tp = ppA.tile([128, 352], F32, name="big")
                        nc.tensor.transpose(tp[:, :qcs], Otm_k[qc][:qcs, 2 * m:2 * m + 2, :].rearrange("p a b -> p (a b)"),
                                            idf32[:qcs, :qcs])
                        nc.scalar.copy(aOTm[:, m, q0:q0 + qcs], tp[:, :qcs])
            else:
                # ---- CLS attention ----
                aCtm = mp.tile([1, 4, 64], F32, name="aCtm")
                acm = mp.tile([128, 4, 3], F16, name="acm")
                for h in range(4):
                    hm, hr = divmod(h, 2)
                    srow = ppS.tile([128, 352], F32, name="srow")
                    nc.tensor.matmul(srow[:1, :257], QTm[hr * 64:hr * 64 + 64, hm, 0:1],
                                     KTm[hr * 64:hr * 64 + 64, hm, :], start=True, stop=True)
                    erow = ap_.tile([1, 257], F32, name="erowm")
                    nc.scalar.activation(erow[:], srow[:1, :257], AF.Exp)
                    rs = ap_.tile([1, 1], F32, name="rsm")
                    nc.vector.reduce_sum(rs[:], erow[:], axis=AX.X)
                    rc = ap_.tile([1, 1], F32, name="rcm1")
                    nc.vector.reciprocal(rc[:], rs[:])
                    acp = ppC.tile([128, 352], F32, name="avp")
                    for kc, (c0, kcs) in enumerate(KCH):
                        nc.tensor.matmul(acp[:kcs, kc:kc + 1], erow[:, c0:c0 + kcs], rc[:],
                                         start=True, stop=True)
                        nc.scalar.copy(acm[:kcs, h, kc:kc + 1], acp[:kcs, kc:kc + 1])
                    av = ppC.tile([128, 352], F32, name="avp")
                    for kc, (c0, kcs) in enumerate(KCH):
                        nc.tensor.matmul(av[:1, h * 64:h * 64 + 64], acm[:kcs, h, kc:kc + 1],
                                         Vtm_k[kc][:kcs, h, :64], start=(kc == 0), stop=(kc == 2))
                    nc.scalar.copy(aCtm[:, h, :], av[:1, h * 64:h * 64 + 64])
                aOTm = mp.tile([128, 2, 1], F16, name="aOTc")
                for m in range(2):
                    tp = ppA.tile([128, 352], F32, name="big")
                    nc.tensor.transpose(tp[:, :1], aCtm[:, 2 * m:2 * m + 2, :].rearrange("p a b -> p (a b)"),
                                        idf32[:1, :1])
                    nc.scalar.copy(aOTm[:, m, :], tp[:, :1])

            # ---- out-proj + z1 + LN1 + x1 ----
            z1m = fp.tile([128, 2, 258], F32R, name="z1c")
            x1m = mp.tile([128, 2, SM], F16, name="x1m")
            for m in range(2):
                ps = ppA.tile([128, 352], F32, name="big")
                for kc in range(2):
                    nc.tensor.matmul(ps[:, :nq], mt_wo_sb[:, kc, li, m * 128:(m + 1) * 128],
                                     aOTm[:, kc, :], start=(kc == 0), stop=(kc == 1))
                nc.vector.scalar_tensor_tensor(z1m[:, m, :nq], ps[:, :nq], mt_bo_sb[:, m:m + 1, li],
                                               xin[:, m, :nq], ALU.add, ALU.add)
            if cls_only:
                ln_std("lnm1c", z1m, 0, 1, F16, [x1m[:, 0, 0:1], x1m[:, 1, 0:1]],
                       mt_g1_sb, mt_b1v_sb, gi=li)
            else:
                ln_fast(z1m, SM, [x1m[:, 0, :], x1m[:, 1, :]], mt_g1_sb, mt_b1v_sb, gi=li)
            # ---- FFN + z2 + LN2 ----
            Hm = fp.tile([128, 16, SM], F16, name="Hc")
            z2m = fp.tile([128, 2, 258], F32R, name="z2c")
            for fm in range(16):
                ps = ppA.tile([128, 352], F32, name="big")
                for kc in range(2):
                    nc.tensor.matmul(ps[:, :nq], mt_w1_sb[:, kc, li, fm * 128:(fm + 1) * 128],
                                     x1m[:, kc, :nq], start=(kc == 0), stop=(kc == 1))
                nc.scalar.activation(Hm[:, fm, :nq], ps[:, :nq], AF.Relu, bias=mt_b1_sb[:, fm:fm + 1, li])
            for m in range(2):
                ps = ppA.tile([128, 352], F32, name="big")
                for fk in range(16):
                    nc.tensor.matmul(ps[:, :nq], mt_w2_sb[:, fk, li, m * 128:(m + 1) * 128],
                                     Hm[:, fk, :nq], start=(fk == 0), stop=(fk == 15))
                nc.vector.scalar_tensor_tensor(z2m[:, m, :nq], ps[:, :nq], mt_b2_sb[:, m:m + 1, li],
                                               x1m[:, m, :nq], ALU.add, ALU.add)
            if cls_only:
                xf = mp.tile([128, 2, 1], F32, name="xf")
                ln_std("lnm2c", z2m, 0, 1, F32, [xf[:, 0, :], xf[:, 1, :]],
                       mt_g2_sb, mt_b2v_sb, gi=li)
            else:
                xnext = mp.tile([128, 2, SM], F16, name="xm1")
                ln_fast(z2m, SM, [xnext[:, 0, :], xnext[:, 1, :]], mt_g2_sb, mt_b2v_sb, gi=li)
                xin = xnext

        # ---- head ----
        ps = ppA.tile([128, 352], F32, name="big")
        for kc in range(2):
            nc.tensor.matmul(ps[:1, :2], xf[:, kc, 0:1], w_head_sb[:, kc, :],
                             start=(kc == 0), stop=(kc == 1))
        osb = mp.tile([1, 2], F32, name="osb")
        nc.vector.tensor_add(osb[:], ps[:1, :2], b_head_sb[:])
        nc.sync.dma_start(out[:], osb[:])


def _build():
    if "nc" in _CACHE:
        return _CACHE["nc"]
    nc = bacc.Bacc("TRN2", target_bir_lowering=False, debug=False, num_devices=8)
    _emit(nc)
    nc.compile()
    _CACHE["nc"] = nc
    return nc


def _prep(inputs):
    f32 = np.float32
    f16 = np.float16
    bf16 = ml_dtypes.bfloat16
    g = lambda k: np.asarray(inputs[k], f32)

    def sb2(a, c, dt=f32):   # [c*128] -> [128, c]
        return np.ascontiguousarray(np.asarray(a, f32).reshape(c, 128).T).astype(dt)

    def sb3(a, kc, dt=f32):  # [kc*128, m] -> [128, kc*m]
        a = np.asarray(a, f32)
        return np.ascontiguousarray(
            a.reshape(kc, 128, -1).transpose(1, 0, 2).reshape(128, -1)).astype(dt)

    def sb4(a, kc, dt=f32):  # [l, kc*128, m] -> [128, kc*l*m]
        a = np.asarray(a, f32)
        l = a.shape[0]
        return np.ascontiguousarray(
            a.reshape(l, kc, 128, -1).transpose(2, 1, 0, 3).reshape(128, -1)).astype(dt)

    def sbb(a, dt=f32):      # [l, c*128] -> [128, c*l]
        a = np.asarray(a, f32)
        l, n = a.shape
        c = n // 128
        return np.ascontiguousarray(
            a.reshape(l, c, 128).transpose(2, 1, 0).reshape(128, -1)).astype(dt)

    pg = np.asarray(inputs["patch_grid"])
    desc = g("desc_texts_grid")
    tbl = g("action_emb_table")
    bw, bb = g("bert_proj_w"), g("bert_proj_b")
    fw, fb = g("fc_w"), g("fc_b")
    assert int(pg[..., 0].max()) <= 1, "action ids exceed {0,1}; kernel fold invalid"

    W_a, W_d, w_anc = fw[:, :AE], fw[:, AE:AE + DE], fw[:, AE + DE]
    com = {}
    f8 = ml_dtypes.float8_e4m3
    com["w_emb"] = np.clip(sb3((W_d @ bw).T * 64.0, 6), -240, 240).astype(f8)
    c0 = W_a @ tbl[0]
    c1 = W_a @ (tbl[1] - tbl[0])
    b0 = fb + c0 + W_d @ bb
    com["w_ex"] = np.ascontiguousarray(np.stack([c1, w_anc, b0]) * 64.0).astype(bf16)
    wq = g("pe_in_w").copy()
    bq = g("pe_in_b").copy()
    wq[:E] *= 128.0 ** -0.5
    bq[:E] *= 128.0 ** -0.5
    com["pe_wqkv"] = sb3(wq.T, 2, f16)
    com["pe_bqkv"] = sb2(bq, 6)
    # patch-encoder CLS query is constant: q = Wq @ patch_cls + bq (scaled);
    # score vector u = Wk^T q (bias const cancels in softmax)
    qv = wq[:E] @ g("patch_cls") + bq[:E]
    Wk = wq[E:2 * E]
    Umat = np.stack([Wk[h * 128:(h + 1) * 128].T @ qv[h * 128:(h + 1) * 128]
                     for h in range(2)], axis=1)  # [E, 2]
    com["ucls"] = sb3(Umat, 2, f16)
    # block-diag mask: M[t, 60 + t//32] = 1; group g slices [60-4g : 124-4g]
    M = np.zeros((128, 124), f16)
    for t in range(128):
        M[t, 60 + t // 32] = 1.0
    com["amask"] = M
    # CLS token folds to a constant per head: e_cls * [v_cls, 1]
    Wv = wq[2 * E:3 * E]
    v_cls = Wv @ g("patch_cls")
    ecv_v = np.zeros((1, 2, 129), f32)
    for h in range(2):
        e_cls = float(np.exp(qv[h * 128:(h + 1) * 128] @ Wk[h * 128:(h + 1) * 128] @ g("patch_cls")))
        ecv_v[0, h, :128] = e_cls * v_cls[h * 128:(h + 1) * 128]
        ecv_v[0, h, 128] = e_cls
    com["ecv"] = ecv_v.reshape(1, 258).astype(f16)
    com["pe_wo"] = sb3(g("pe_out_w").T, 2, f16)
    com["pe_bo"] = sb2(g("pe_out_b") + g("patch_cls"), 2)  # CLS residual folded
    com["pe_g1"] = sb2(g("pe_ln1_g"), 2)
    com["pe_b1v"] = sb2(g("pe_ln1_b"), 2)
    com["pe_w1"] = sb3(g("pe_w1").T, 2, f16)
    com["pe_b1"] = sb2(g("pe_b1"), 16)
    com["pe_w2"] = sb3(g("pe_w2").T, 16, f16)
    com["pe_b2"] = sb2(g("pe_b2"), 2)
    com["pe_g2"] = sb2(g("pe_ln2_g"), 2)
    com["pe_b2v"] = sb2(g("pe_ln2_b"), 2)
    pos = (g("user_pos")[:U][:, None, :] + g("time_pos")[None, :T, :]).reshape(256, E)
    com["posb"] = sb3(pos.T, 2)
    com["pcls"] = sb2(g("patch_cls"), 2)
    com["mcls"] = sb2(g("main_cls"), 2)
    mwq = g("mt_in_w").copy()
    mbq = g("mt_in_b").copy()
    mwq[:, :E] *= 64.0 ** -0.5
    mbq[:, :E] *= 64.0 ** -0.5
    com["mt_wqkv"] = sb4(mwq.transpose(0, 2, 1), 2, f16)
    com["mt_bqkv"] = sbb(mbq)
    com["mt_wo"] = sb4(g("mt_out_w").transpose(0, 2, 1), 2, f16)
    com["mt_bo"] = sbb(g("mt_out_b"))
    com["mt_g1"] = sbb(g("mt_ln1_g"))
    com["mt_b1v"] = sbb(g("mt_ln1_b"))
    com["mt_w1"] = sb4(g("mt_w1").transpose(0, 2, 1), 2, f16)
    com["mt_b1"] = sbb(g("mt_b1"))
    com["mt_w2"] = sb4(g("mt_w2").transpose(0, 2, 1), 16, f16)
    com["mt_b2"] = sbb(g("mt_b2"))
    com["mt_g2"] = sbb(g("mt_ln2_g"))
    com["mt_b2v"] = sbb(g("mt_ln2_b"))
    com["w_head"] = sb3(g("head_w").T, 2)
    com["b_head"] = g("head_b").reshape(1, 2)

    in_maps = []
    for b in range(B):
        ids = pg[b, ..., 0].reshape(8192).astype(f32)
        anc = pg[b, ..., 1].reshape(8192).astype(f32)
        m = dict(com)
        m["extras"] = np.ascontiguousarray(np.stack([ids, anc, np.ones(8192, f32)])).astype(bf16)
        dT = desc[b].reshape(8192, DESC).T  # [768, 8192]
        m["descT"] = np.clip(np.ascontiguousarray(
            dT.reshape(6, 128, 8192).transpose(1, 0, 2)), -240, 240).astype(
            ml_dtypes.float8_e4m3)
        in_maps.append(m)
    return in_maps


def kernel(**inputs):
    nc = _build()
    in_maps = _prep(inputs)
    res = run_bass_kernel_spmd(nc, in_maps, core_ids=list(range(8)))
    return np.stack([res.results[i]["out"][0] for i in range(B)]).astype(np.float32)
